# revision 12
# baseline (speedup 1.0000x reference)
"""Trainium2 Bass kernel for nn_AudioModel segment_reduce.

Reference computation (per batch row b):
  - frames t < audio_lengths[b] are valid
  - consecutive runs of equal phoneme_ids form segments
  - feat[b] = mean over segments of (mean over frames in segment of h[b,t,:])
  - logit[b] = feat[b] @ W.T + bias

Algebraic collapse: feat[b] = sum_t w[b,t] * h[b,t,:] with
  w[b,t] = valid[b,t] / (run_len(b, run_of(t)) * n_runs[b])
so  logit[b] = sum_t w[b,t] * (h[b,t,:] . W) + bias.

The per-frame weights w depend only on the tiny phoneme_ids/audio_lengths
tensors and are computed on host. hidden_states is the only heavy tensor
and the kernel is memory-bound on streaming it, so the device stream is
minimized:
  - fp16 storage (host casts) — halves traffic; max rel err ~4e-3 vs the
    2e-2 gate (bf16/fp8 fail it);
  - frames past audio_length have zero weight and are never shipped: ALL
    valid frames are concatenated into one global stream that is cut into
    8 equal per-core streams (a row split by a cut contributes partial
    sums that the host adds). Every core moves the same minimal byte
    count and runs one identical program.
Which batch row each frame belongs to is data, not program structure: the
per-chunk segment weights live in an M=SEGS column block of a weight
matrix, so chunk->row mapping needs no per-core code.

The ~410 GB/s HWDGE stream (4-chunk ~0.8MB pieces alternating the two
rings, partition-major so each piece is 128 long contiguous lines) is
consumed by BOTH compute engines so neither can pace it (the PE alone
oscillates across the HAM 1.2/2.4 GHz clock gate and throttles the
stream): 3 of every 8 chunks contract on the PE as (128,SEGS)x(128,512|
256) fp16 matmuls into a (SEGS,C) fp32 PSUM group; the other 5 contract
on the DVE — one fused scalar_tensor_tensor multiplies the chunk by a
partition-replicated W and row-reduces to per-frame scores (the W-dot),
then a near-free N=1 PE matmul applies the per-segment frame weights
into a (SEGS,1) PSUM column. Two final fused DVE ops apply W to the PE
path and add both paths; weights ride the HWDGE rings ahead of the
stream (SWDGE is ~90 GB/s — only the tiny classifier vector goes there).
"""

import numpy as np

B, T, C = 128, 1496, 768
NCORES = 8
PF = 128                   # frames per chunk
CPT = 8                    # chunks per DMA piece
DVE_MOD = (1, 5)           # chunks c with c%8 in this set contract on DVE

_CACHE = {}


def _frame_weights(phoneme_ids, audio_lengths):
    """w[b,t] = valid / (run_length(run containing t) * n_runs[b]); 0 if invalid."""
    pid = np.asarray(phoneme_ids)
    L = np.asarray(audio_lengths).astype(np.int64)
    t = np.arange(T)
    valid = t[None, :] < L[:, None]                               # (B, T)
    change = pid[:, 1:] != pid[:, :-1]
    boundary = np.concatenate([np.ones((B, 1), bool), change], axis=1) & valid
    seg = np.cumsum(boundary, axis=1) - 1
    np.maximum(seg, 0, out=seg)                                   # (B, T)
    gid = (seg + np.arange(B, dtype=np.int64)[:, None] * T).ravel()
    cnt = np.bincount(gid, weights=valid.ravel().astype(np.float64), minlength=B * T)
    cnt_t = cnt[gid].reshape(B, T)                                # run length per frame
    n_runs = boundary.sum(axis=1).astype(np.float64)              # (B,)
    w = np.where(valid, 1.0 / (np.maximum(cnt_t, 1.0) * n_runs[:, None]), 0.0)
    return w.astype(np.float32)


def _plan(audio_lengths):
    """Cut the global valid-frame stream into 8 equal per-core streams.

    Returns per-core segment lists [(row, lo, hi), ...] plus the uniform
    NCHUNK (128-frame chunks per core) and SEGS (max segments per core).
    """
    L = np.minimum(np.asarray(audio_lengths).astype(np.int64), T)
    cum = np.concatenate([[0], np.cumsum(L)])
    F = int(cum[-1])
    cuts = [(F * i) // NCORES for i in range(NCORES + 1)]
    segs = []
    for i in range(NCORES):
        lo_g, hi_g = cuts[i], cuts[i + 1]
        b0 = int(np.searchsorted(cum, lo_g, side="right")) - 1
        s = []
        g = lo_g
        b = b0
        while g < hi_g:
            e = min(int(cum[b + 1]), hi_g)
            s.append((b, g - int(cum[b]), e - int(cum[b])))
            g = e
            b += 1
        segs.append(s)
    SEGS = max(len(s) for s in segs)
    FRC = max(cuts[i + 1] - cuts[i] for i in range(NCORES))
    NCHUNK = -(-FRC // PF)
    return segs, NCHUNK, SEGS


def _build_program(NCHUNK, SEGS):
    import concourse.bacc as bacc
    import concourse.tile as tile
    from concourse import mybir

    f16 = mybir.dt.float16
    f32 = mybir.dt.float32

    nc = bacc.Bacc("TRN2", target_bir_lowering=False, debug=False)
    h = nc.dram_tensor("h", [128, NCHUNK * C], f16, kind="ExternalInput").ap()
    wt = nc.dram_tensor("wt", [128, NCHUNK * SEGS], f16, kind="ExternalInput").ap()
    wv = nc.dram_tensor("wv", [SEGS, C], f32, kind="ExternalInput").ap()
    wb = nc.dram_tensor("wb", [128, C], f16, kind="ExternalInput").ap()
    out = nc.dram_tensor("out", [SEGS, 1], f32, kind="ExternalOutput").ap()

    # Taper: final 8 chunks ship as 4+2+2 pieces and lean harder on the
    # DVE, so almost no compute trails the last DMA.
    def is_dve(c):
        return c % 8 in DVE_MOD or (NCHUNK - c <= 8 and c % 8 in (7, 2, 3))

    pieces = []
    off = 0
    while off < NCHUNK:
        rem = NCHUNK - off
        if off == 0 and NCHUNK >= 16:
            n = 2      # front taper 2+2+4: compute starts ~5us sooner
        elif off == 2 and NCHUNK >= 16:
            n = 2
        elif off == 4 and NCHUNK >= 16:
            n = 4
        elif rem > 8:
            n = min(CPT, rem - 8)
        elif rem > 4:
            n = rem - 4
        else:
            n = min(2, rem)
        pieces.append((off, n))
        off += n
    pe_chunks = [c for c in range(NCHUNK) if not is_dve(c)]
    dve_chunks = [c for c in range(NCHUNK) if is_dve(c)]

    with tile.TileContext(nc) as tc:
        with (
            tc.tile_pool(name="hp", bufs=6) as hp,
            tc.tile_pool(name="const", bufs=1) as cp,
            tc.tile_pool(name="ps", bufs=1, space="PSUM") as pp,
            tc.tile_pool(name="qp", bufs=1, space="PSUM") as qp,
            tc.tile_pool(name="sco", bufs=3) as scop,
            tc.tile_pool(name="scol", bufs=6) as scolp,
            tc.tile_pool(name="fin", bufs=1) as fp,
        ):
            # Weights ride the fast HWDGE rings ahead of the h stream (the
            # SWDGE path runs ~90 GB/s and would gate the first matmul).
            wtile = cp.tile([128, NCHUNK * SEGS], f16)
            half = (NCHUNK * SEGS) // 2
            nc.sync.dma_start(wtile[:, :half], wt[:, :half])
            nc.scalar.dma_start(wtile[:, half:], wt[:, half:])
            wbt = cp.tile([128, C], f16)
            nc.sync.dma_start(wbt[:], wb)
            wvt = cp.tile([SEGS, C], f32)
            nc.gpsimd.dma_start(wvt[:], wv)
            logits = cp.tile([SEGS, 1], f32)

            ps = pp.tile([SEGS, C], f32)
            qcol = qp.tile([SEGS, 1], f32)
            # (chunk, scol tile) pairs whose N=1 matmul is emitted one piece
            # later, so the PE never waits on a just-computed score column.
            pending = []

            def flush_pending():
                for c_, scol_ in pending:
                    nc.tensor.matmul(
                        qcol[:], wtile[:, c_ * SEGS : (c_ + 1) * SEGS], scol_[:],
                        start=(c_ == dve_chunks[0]), stop=(c_ == dve_chunks[-1]),
                    )
                pending.clear()

            for pc, (poff, n) in enumerate(pieces):
                ring = nc.sync if pc % 2 == 0 else nc.scalar
                ht = hp.tile([128, CPT * C], f16, tag="h")
                ring.dma_start(ht[:, : n * C], h[:, poff * C : (poff + n) * C])
                for j in range(n):
                    c = poff + j
                    if is_dve(c):
                        sco = scop.tile([128, C], f16, tag="sco")
                        scol = scolp.tile([128, 1], f16, tag="scol")
                        nc.vector.scalar_tensor_tensor(
                            out=sco[:],
                            in0=ht[:, j * C : (j + 1) * C],
                            scalar=1.0,
                            in1=wbt[:],
                            op0=mybir.AluOpType.mult,
                            op1=mybir.AluOpType.mult,
                            accum_out=scol[:],
                        )
                        pending.append((c, scol))
                for j in range(n):
                    c = poff + j
                    if is_dve(c):
                        continue
                    lw = wtile[:, c * SEGS : (c + 1) * SEGS]
                    nc.tensor.matmul(
                        ps[:, :512], lw, ht[:, j * C : j * C + 512],
                        start=(c == pe_chunks[0]), stop=(c == pe_chunks[-1]),
                    )
                    nc.tensor.matmul(
                        ps[:, 512:], lw, ht[:, j * C + 512 : (j + 1) * C],
                        start=(c == pe_chunks[0]), stop=(c == pe_chunks[-1]),
                    )
                if pc == len(pieces) - 1:
                    flush_pending()
                else:
                    # emit N=1 matmuls for scols of pieces before this one
                    done = [x for x in pending if x[0] < poff]
                    for c_, scol_ in done:
                        nc.tensor.matmul(
                            qcol[:], wtile[:, c_ * SEGS : (c_ + 1) * SEGS], scol_[:],
                            start=(c_ == dve_chunks[0]), stop=(c_ == dve_chunks[-1]),
                        )
                    pending[:] = [x for x in pending if x[0] >= poff]

            sc = fp.tile([SEGS, C], f32, tag="sc")
            t1 = fp.tile([SEGS, 1], f32, tag="t1")
            nc.vector.scalar_tensor_tensor(
                out=sc[:],
                in0=ps[:],
                scalar=1.0,
                in1=wvt[:],
                op0=mybir.AluOpType.mult,
                op1=mybir.AluOpType.mult,
                accum_out=t1[:],
            )
            nc.vector.scalar_tensor_tensor(
                out=logits[:],
                in0=t1[:],
                scalar=1.0,
                in1=qcol[:],
                op0=mybir.AluOpType.mult,
                op1=mybir.AluOpType.add,
            )
            nc.sync.dma_start(out, logits[:])

    nc.compile()
    return nc


def _get_program(NCHUNK, SEGS):
    key = (NCHUNK, SEGS)
    if key not in _CACHE:
        _CACHE[key] = _build_program(*key)
    return _CACHE[key]


def _pack(hidden, w, segs, NCHUNK, SEGS):
    """Per-core packed fp16 frame stream (partition-major) and weight matrix."""
    h_maps, wt_maps = [], []
    for s in segs:
        hbuf = np.zeros((NCHUNK * PF, C), dtype=np.float16)
        q = 0
        wvals = []
        sids = []
        for sid, (b, lo, hi) in enumerate(s):
            n = hi - lo
            hbuf[q : q + n] = hidden[b, lo:hi]
            wvals.append(w[b, lo:hi])
            sids.append(np.full(n, sid, dtype=np.int64))
            q += n
        h_maps.append(
            np.ascontiguousarray(
                hbuf.reshape(NCHUNK, PF, C).transpose(1, 0, 2)
            ).reshape(128, NCHUNK * C)
        )
        wvals = np.concatenate(wvals).astype(np.float16)
        sids = np.concatenate(sids)
        pos = np.arange(q)
        wbuf = np.zeros((128, NCHUNK * SEGS), dtype=np.float16)
        wbuf[pos % PF, (pos // PF) * SEGS + sids] = wvals
        wt_maps.append(wbuf)
    return h_maps, wt_maps


def _run(inputs, trace=False):
    from concourse.bass_utils import run_bass_kernel_spmd

    hidden = np.asarray(inputs["hidden_states"], dtype=np.float32)
    W = np.ascontiguousarray(np.asarray(inputs["W"], dtype=np.float32)).reshape(1, C)
    bias = np.asarray(inputs["b"], dtype=np.float32)
    w = _frame_weights(inputs["phoneme_ids"], inputs["audio_lengths"])
    segs, NCHUNK, SEGS = _plan(inputs["audio_lengths"])
    h_maps, wt_maps = _pack(hidden, w, segs, NCHUNK, SEGS)
    wv_rep = np.ascontiguousarray(np.repeat(W, SEGS, axis=0))
    wb_rep = np.ascontiguousarray(np.repeat(W.astype(np.float16), 128, axis=0))

    in_maps = [
        {"h": h_maps[i], "wt": wt_maps[i], "wv": wv_rep, "wb": wb_rep}
        for i in range(NCORES)
    ]

    nc = _get_program(NCHUNK, SEGS)
    res = run_bass_kernel_spmd(nc, in_maps, list(range(NCORES)), trace=trace)
    logit = np.zeros((B, 1), dtype=np.float64)
    for i in range(NCORES):
        o = res.results[i]["out"]
        for sid, (b, lo, hi) in enumerate(segs[i]):
            logit[b, 0] += float(o[sid, 0])
    logit = logit.astype(np.float32) + bias[None, :]
    return logit.astype(np.float32), res


def kernel(**inputs):
    return _run(inputs, trace=False)[0]


# revision 13
# speedup vs baseline: 1.1701x; 1.1701x over previous
"""Trainium2 Bass kernel for nn_AudioModel segment_reduce.

Reference computation (per batch row b):
  - frames t < audio_lengths[b] are valid
  - consecutive runs of equal phoneme_ids form segments
  - feat[b] = mean over segments of (mean over frames in segment of h[b,t,:])
  - logit[b] = feat[b] @ W.T + bias

Algebraic collapse: feat[b] = sum_t w[b,t] * h[b,t,:] with
  w[b,t] = valid[b,t] / (run_len(b, run_of(t)) * n_runs[b])
so  logit[b] = sum_t w[b,t] * (h[b,t,:] . W) + bias.

The per-frame weights w depend only on the tiny phoneme_ids/audio_lengths
tensors and are computed on host. hidden_states is the only heavy tensor
and the kernel is memory-bound on streaming it, so the device stream is
minimized:
  - fp16 storage (host casts) — halves traffic; max rel err ~4e-3 vs the
    2e-2 gate (bf16/fp8 fail it);
  - frames past audio_length have zero weight and are never shipped: ALL
    valid frames are concatenated into one global stream that is cut into
    8 equal per-core streams (a row split by a cut contributes partial
    sums that the host adds). Every core moves the same minimal byte
    count and runs one identical program.
Which batch row each frame belongs to is data, not program structure: the
per-chunk segment weights live in an M=SEGS column block of a weight
matrix, so chunk->row mapping needs no per-core code.

The ~410 GB/s HWDGE stream (4-chunk ~0.8MB pieces alternating the two
rings, partition-major so each piece is 128 long contiguous lines) is
consumed by BOTH compute engines so neither can pace it (the PE alone
oscillates across the HAM 1.2/2.4 GHz clock gate and throttles the
stream): 3 of every 8 chunks contract on the PE as (128,SEGS)x(128,512|
256) fp16 matmuls into a (SEGS,C) fp32 PSUM group; the other 5 contract
on the DVE — one fused scalar_tensor_tensor multiplies the chunk by a
partition-replicated W and row-reduces to per-frame scores (the W-dot),
then a near-free N=1 PE matmul applies the per-segment frame weights
into a (SEGS,1) PSUM column. Two final fused DVE ops apply W to the PE
path and add both paths; weights ride the HWDGE rings ahead of the
stream (SWDGE is ~90 GB/s — only the tiny classifier vector goes there).
"""

import numpy as np

B, T, C = 128, 1496, 768
NCORES = 8
PF = 128                   # frames per chunk
CPT = 8                    # chunks per DMA piece
DVE_MOD = (1, 5)           # chunks c with c%8 in this set contract on DVE

_CACHE = {}


def _frame_weights(phoneme_ids, audio_lengths):
    """w[b,t] = valid / (run_length(run containing t) * n_runs[b]); 0 if invalid."""
    pid = np.asarray(phoneme_ids)
    L = np.asarray(audio_lengths).astype(np.int64)
    t = np.arange(T)
    valid = t[None, :] < L[:, None]                               # (B, T)
    change = pid[:, 1:] != pid[:, :-1]
    boundary = np.concatenate([np.ones((B, 1), bool), change], axis=1) & valid
    seg = np.cumsum(boundary, axis=1) - 1
    np.maximum(seg, 0, out=seg)                                   # (B, T)
    gid = (seg + np.arange(B, dtype=np.int64)[:, None] * T).ravel()
    cnt = np.bincount(gid, weights=valid.ravel().astype(np.float64), minlength=B * T)
    cnt_t = cnt[gid].reshape(B, T)                                # run length per frame
    n_runs = boundary.sum(axis=1).astype(np.float64)              # (B,)
    w = np.where(valid, 1.0 / (np.maximum(cnt_t, 1.0) * n_runs[:, None]), 0.0)
    return w.astype(np.float32)


def _plan(audio_lengths):
    """Cut the global valid-frame stream into 8 equal per-core streams.

    Returns per-core segment lists [(row, lo, hi), ...] plus the uniform
    NCHUNK (128-frame chunks per core) and SEGS (max segments per core).
    """
    L = np.minimum(np.asarray(audio_lengths).astype(np.int64), T)
    cum = np.concatenate([[0], np.cumsum(L)])
    F = int(cum[-1])
    cuts = [(F * i) // NCORES for i in range(NCORES + 1)]
    segs = []
    for i in range(NCORES):
        lo_g, hi_g = cuts[i], cuts[i + 1]
        b0 = int(np.searchsorted(cum, lo_g, side="right")) - 1
        s = []
        g = lo_g
        b = b0
        while g < hi_g:
            e = min(int(cum[b + 1]), hi_g)
            s.append((b, g - int(cum[b]), e - int(cum[b])))
            g = e
            b += 1
        segs.append(s)
    SEGS = max(len(s) for s in segs)
    FRC = max(cuts[i + 1] - cuts[i] for i in range(NCORES))
    NCHUNK = -(-FRC // PF)
    return segs, NCHUNK, SEGS


def _build_program(NCHUNK, SEGS):
    import concourse.bacc as bacc
    import concourse.tile as tile
    from concourse import mybir

    f16 = mybir.dt.float16
    f32 = mybir.dt.float32

    nc = bacc.Bacc("TRN2", target_bir_lowering=False, debug=False)
    h = nc.dram_tensor("h", [128, NCHUNK * C], f16, kind="ExternalInput").ap()
    wt = nc.dram_tensor("wt", [128, NCHUNK * SEGS], f16, kind="ExternalInput").ap()
    wv = nc.dram_tensor("wv", [SEGS, C], f32, kind="ExternalInput").ap()
    wb = nc.dram_tensor("wb", [128, C], f16, kind="ExternalInput").ap()
    out = nc.dram_tensor("out", [SEGS, 1], f32, kind="ExternalOutput").ap()

    # Taper: final 8 chunks ship as 4+2+2 pieces and lean harder on the
    # DVE, so almost no compute trails the last DMA.
    def is_dve(c):
        return c % 8 in DVE_MOD or (NCHUNK - c <= 8 and c % 8 in (7, 2, 3))

    pieces = []
    off = 0
    while off < NCHUNK:
        rem = NCHUNK - off
        if rem > 8:
            n = min(CPT, rem - 8)
        elif rem > 4:
            n = rem - 4
        else:
            n = min(2, rem)
        pieces.append((off, n))
        off += n
    pe_chunks = [c for c in range(NCHUNK) if not is_dve(c)]
    dve_chunks = [c for c in range(NCHUNK) if is_dve(c)]

    with tile.TileContext(nc) as tc:
        with (
            tc.tile_pool(name="hp", bufs=6) as hp,
            tc.tile_pool(name="const", bufs=1) as cp,
            tc.tile_pool(name="ps", bufs=1, space="PSUM") as pp,
            tc.tile_pool(name="qp", bufs=1, space="PSUM") as qp,
            tc.tile_pool(name="sco", bufs=3) as scop,
            tc.tile_pool(name="scol", bufs=6) as scolp,
            tc.tile_pool(name="fin", bufs=1) as fp,
        ):
            # Weights ride the fast HWDGE rings ahead of the h stream (the
            # SWDGE path runs ~90 GB/s and would gate the first matmul).
            wtile = cp.tile([128, NCHUNK * SEGS], f16)
            half = (NCHUNK * SEGS) // 2
            nc.sync.dma_start(wtile[:, :half], wt[:, :half])
            nc.scalar.dma_start(wtile[:, half:], wt[:, half:])
            wbt = cp.tile([128, C], f16)
            nc.sync.dma_start(wbt[:], wb)
            wvt = cp.tile([SEGS, C], f32)
            nc.gpsimd.dma_start(wvt[:], wv)
            logits = cp.tile([SEGS, 1], f32)

            ps = pp.tile([SEGS, C], f32)
            qcol = qp.tile([SEGS, 1], f32)
            # (chunk, scol tile) pairs whose N=1 matmul is emitted one piece
            # later, so the PE never waits on a just-computed score column.
            pending = []

            def flush_pending():
                for c_, scol_ in pending:
                    nc.tensor.matmul(
                        qcol[:], wtile[:, c_ * SEGS : (c_ + 1) * SEGS], scol_[:],
                        start=(c_ == dve_chunks[0]), stop=(c_ == dve_chunks[-1]),
                    )
                pending.clear()

            for pc, (poff, n) in enumerate(pieces):
                ring = nc.sync if pc % 2 == 0 else nc.scalar
                ht = hp.tile([128, CPT * C], f16, tag="h")
                ring.dma_start(ht[:, : n * C], h[:, poff * C : (poff + n) * C])
                for j in range(n):
                    c = poff + j
                    if is_dve(c):
                        sco = scop.tile([128, C], f16, tag="sco")
                        scol = scolp.tile([128, 1], f16, tag="scol")
                        nc.vector.scalar_tensor_tensor(
                            out=sco[:],
                            in0=ht[:, j * C : (j + 1) * C],
                            scalar=1.0,
                            in1=wbt[:],
                            op0=mybir.AluOpType.mult,
                            op1=mybir.AluOpType.mult,
                            accum_out=scol[:],
                        )
                        pending.append((c, scol))
                for j in range(n):
                    c = poff + j
                    if is_dve(c):
                        continue
                    lw = wtile[:, c * SEGS : (c + 1) * SEGS]
                    nc.tensor.matmul(
                        ps[:, :512], lw, ht[:, j * C : j * C + 512],
                        start=(c == pe_chunks[0]), stop=(c == pe_chunks[-1]),
                    )
                    nc.tensor.matmul(
                        ps[:, 512:], lw, ht[:, j * C + 512 : (j + 1) * C],
                        start=(c == pe_chunks[0]), stop=(c == pe_chunks[-1]),
                    )
                if pc == len(pieces) - 1:
                    flush_pending()
                else:
                    # emit N=1 matmuls for scols of pieces before this one
                    done = [x for x in pending if x[0] < poff]
                    for c_, scol_ in done:
                        nc.tensor.matmul(
                            qcol[:], wtile[:, c_ * SEGS : (c_ + 1) * SEGS], scol_[:],
                            start=(c_ == dve_chunks[0]), stop=(c_ == dve_chunks[-1]),
                        )
                    pending[:] = [x for x in pending if x[0] >= poff]

            sc = fp.tile([SEGS, C], f32, tag="sc")
            t1 = fp.tile([SEGS, 1], f32, tag="t1")
            nc.vector.scalar_tensor_tensor(
                out=sc[:],
                in0=ps[:],
                scalar=1.0,
                in1=wvt[:],
                op0=mybir.AluOpType.mult,
                op1=mybir.AluOpType.mult,
                accum_out=t1[:],
            )
            nc.vector.scalar_tensor_tensor(
                out=logits[:],
                in0=t1[:],
                scalar=1.0,
                in1=qcol[:],
                op0=mybir.AluOpType.mult,
                op1=mybir.AluOpType.add,
            )
            nc.sync.dma_start(out, logits[:])

    nc.compile()
    return nc


def _get_program(NCHUNK, SEGS):
    key = (NCHUNK, SEGS)
    if key not in _CACHE:
        _CACHE[key] = _build_program(*key)
    return _CACHE[key]


def _pack(hidden, w, segs, NCHUNK, SEGS):
    """Per-core packed fp16 frame stream (partition-major) and weight matrix."""
    h_maps, wt_maps = [], []
    for s in segs:
        hbuf = np.zeros((NCHUNK * PF, C), dtype=np.float16)
        q = 0
        wvals = []
        sids = []
        for sid, (b, lo, hi) in enumerate(s):
            n = hi - lo
            hbuf[q : q + n] = hidden[b, lo:hi]
            wvals.append(w[b, lo:hi])
            sids.append(np.full(n, sid, dtype=np.int64))
            q += n
        h_maps.append(
            np.ascontiguousarray(
                hbuf.reshape(NCHUNK, PF, C).transpose(1, 0, 2)
            ).reshape(128, NCHUNK * C)
        )
        wvals = np.concatenate(wvals).astype(np.float16)
        sids = np.concatenate(sids)
        pos = np.arange(q)
        wbuf = np.zeros((128, NCHUNK * SEGS), dtype=np.float16)
        wbuf[pos % PF, (pos // PF) * SEGS + sids] = wvals
        wt_maps.append(wbuf)
    return h_maps, wt_maps


def _run(inputs, trace=False):
    from concourse.bass_utils import run_bass_kernel_spmd

    hidden = np.asarray(inputs["hidden_states"], dtype=np.float32)
    W = np.ascontiguousarray(np.asarray(inputs["W"], dtype=np.float32)).reshape(1, C)
    bias = np.asarray(inputs["b"], dtype=np.float32)
    w = _frame_weights(inputs["phoneme_ids"], inputs["audio_lengths"])
    segs, NCHUNK, SEGS = _plan(inputs["audio_lengths"])
    h_maps, wt_maps = _pack(hidden, w, segs, NCHUNK, SEGS)
    wv_rep = np.ascontiguousarray(np.repeat(W, SEGS, axis=0))
    wb_rep = np.ascontiguousarray(np.repeat(W.astype(np.float16), 128, axis=0))

    in_maps = [
        {"h": h_maps[i], "wt": wt_maps[i], "wv": wv_rep, "wb": wb_rep}
        for i in range(NCORES)
    ]

    nc = _get_program(NCHUNK, SEGS)
    res = run_bass_kernel_spmd(nc, in_maps, list(range(NCORES)), trace=trace)
    logit = np.zeros((B, 1), dtype=np.float64)
    for i in range(NCORES):
        o = res.results[i]["out"]
        for sid, (b, lo, hi) in enumerate(segs[i]):
            logit[b, 0] += float(o[sid, 0])
    logit = logit.astype(np.float32) + bias[None, :]
    return logit.astype(np.float32), res


def kernel(**inputs):
    return _run(inputs, trace=False)[0]
